# revision 22
# baseline (speedup 1.0000x reference)
"""Trainium2 Bass kernel for nn_CustomCNNLayer_84559316124470.

The reference computes, per batch b:
    win[b,c,s,m]   = xp[b,c,s+m]                    (xp = x padded with K-1 zeros)
    xw[b,c,s,m,l]  = win[b,c,s,m] * stft_w[l,m]
    xr             = xw.reshape(b, c*K*NK, s)       (raw row-major reshape)
    out            = relu(conv_w @ xr + bias)       (1x1 conv over channels)

Because K*NK == S/2 == 2048, the raw reshape maps
    xr[b, c*2048 + q, p*2048 + m*32 + l] = xp[b, c, 2q+p+m] * stft_w[l, m]
(with s = 2q+p). Hence, with h[b,o,r] = sum_{c,q} conv_w[o, c*2048+q] * xp[b,c,2q+r]
(r in [0, 65)):
    out[b, o, p*2048 + m*32 + l] = relu(stft_w[l,m] * h[b,o,p+m] + bias[o])

So the dense 8.6 GMAC/batch matmul collapses to a (512x4096)@(4096x65)
strided correlation (tensor engine) plus a per-element broadcast expansion
(vector/gpsimd engines) and bias+ReLU (scalar/vector engines).

Sharding: output channels o split across the 8 cores (64 rows each);
window matrices replicated. No collectives.

Precision: mm1 runs on the PE in bf16. With PASSES=3 the fp32 operands are
split hi/lo into bf16 pairs and three accumulating matmuls recover ~fp32
accuracy (error ~1e-5 rel.); fp32 PE matmuls run in multi-pass LOW_HIGH
mode and are not competitive.

Raw (non-Tile) implementation: hand-placed semaphores keep the setup and
teardown overhead minimal (Tile's vector-clock epilogue resets every
semaphore individually, ~10us).
"""

import numpy as np
import ml_dtypes

import concourse.bass as bass
from concourse import bacc, mybir
from concourse.bass_utils import run_bass_kernel_spmd

B, C, S = 4, 2, 4096
K, NK, OUT = 64, 32, 512
Q = K * NK            # 2048 == S // 2
R = K + 1             # 65 shift taps
NCORES = 8
OSH = OUT // NCORES   # 64 output channels per core
KT = 32               # contraction tiles of 128 over c*Q = 4096
W260 = B * R          # per-kt rhs free dim: [z=0 | z=1] x [pair 0 | pair 1] x r
NCH = 8               # DMA chunks over kt
KTC = KT // NCH
CHUNK_W = KTC * (2 * OSH + W260)    # bf16 stream: [ch | cl | xh]
CHUNK8_W = KTC * (OSH + W260)       # fp8 stream:  [c8 | xl9]
CSC, XSC = 16.0, 512.0              # fp8 encode scales; product scale 2^13
F32 = mybir.dt.float32
BF16 = mybir.dt.bfloat16
F8 = mybir.dt.float8e4
GROUPS = [(0, 0), (0, 1), (1, 0), (1, 1)]  # (pr, p)
# expansion work units (group, col0, col1): first group split finer so the
# output DMA stream starts as early as possible
UNITS = (
    [(0, i * 512, (i + 1) * 512) for i in range(4)]
    + [(g, hf * 1024, (hf + 1) * 1024) for g in (1, 2, 3) for hf in range(2)]
)

_PROGRAM = None
_LAST_RESULTS = None


def _build_program():
    nc = bacc.Bacc("TRN2", target_bir_lowering=False, debug=False)
    wbuf = nc.dram_tensor("wbuf", [128, NCH * CHUNK_W], BF16, kind="ExternalInput")
    # fp8 payload travels as uint8 (PJRT lacks float8_e4m3 support) and is
    # bitcast to fp8 at the matmul APs
    wbuf8 = nc.dram_tensor(
        "wbuf8", [128, NCH * CHUNK8_W], mybir.dt.uint8, kind="ExternalInput"
    )
    trow = nc.dram_tensor("trow", [1, Q], F32, kind="ExternalInput")
    bias2 = nc.dram_tensor("bias2", [128, 1], F32, kind="ExternalInput")
    ident = nc.dram_tensor("ident", [OSH, OSH], F32, kind="ExternalInput")
    out = nc.dram_tensor("out", [2 * 128, S], F32, kind="ExternalOutput")

    from contextlib import ExitStack

    with ExitStack() as ctx:
        e = ctx.enter_context
        wt = [e(nc.sbuf_tensor(f"wt{c}", [128, CHUNK_W], BF16)) for c in range(NCH)]
        wt8 = [
            e(nc.sbuf_tensor(f"wt8{c}", [128, CHUNK8_W], mybir.dt.uint8))
            for c in range(NCH)
        ]
        T_sb = e(nc.sbuf_tensor("T_sb", [128, Q], F32))
        b_sb = e(nc.sbuf_tensor("b_sb", [128, 1], F32))
        h_sb = e(nc.sbuf_tensor("h_sb", [OSH, W260], F32))
        id_sb = e(nc.sbuf_tensor("id_sb", [OSH, OSH], F32))
        tmp = [e(nc.sbuf_tensor(f"tmp{g}", [128, Q], F32)) for g in range(4)]
        o_sb = [e(nc.sbuf_tensor(f"o{g}", [128, Q], F32)) for g in range(4)]
        # pad h_ps to a full 2KB PSUM bank so h2_ps lands in its own bank
        h_ps_full = e(nc.psum_tensor("h_ps", [OSH, 512], F32))
        h_ps = h_ps_full[:, :W260]
        corr_ps_full = e(nc.psum_tensor("corr_ps", [OSH, 512], F32))
        corr_ps = corr_ps_full[:, :W260]
        h2_ps_full = e(nc.psum_tensor("h2_ps", [128, 512], F32))
        h2_ps = h2_ps_full[:, : 2 * R]

        sin = [e(nc.semaphore(f"sin{c}")) for c in range(NCH)]  # bf16 chunk DMAs
        sin8 = [e(nc.semaphore(f"si8{c}")) for c in range(NCH)]  # fp8 chunk DMAs
        sTa = e(nc.semaphore("sTa"))    # T broadcast DMA
        sTb = e(nc.semaphore("sTb"))    # bias DMA
        sid = e(nc.semaphore("sid"))    # identity DMA
        spe = e(nc.semaphore("spe"))    # mm1 done
        scp = e(nc.semaphore("scp"))    # h combine done
        scpa = e(nc.semaphore("scpa"))  # h psum->sbuf copy done
        sh2 = e(nc.semaphore("sh2"))    # h2 redistribution DMAs
        stt = e(nc.semaphore("stt"))    # DVE multiplies done (groups 0-2)
        sact = e(nc.semaphore("sact"))  # ACT relu done (groups 0-2)
        sout = e(nc.semaphore("sout"))  # out DMAs
        sems = sin + sin8 + [sTa, sTb, sid, spe, scp, scpa, sh2, stt, sact, sout]

        def exp_aps_unit(g, u0, u1):
            pr, p = GROUPS[g]
            m0, nm = u0 // NK, (u1 - u0) // NK
            off = pr * R + p + m0
            h_exp = (
                h2_ps[:, off : off + nm].unsqueeze(2).to_broadcast((128, nm, NK))
            )
            return (
                tmp[g][:, u0:u1].rearrange("a (m l) -> a m l", l=NK),
                h_exp,
                T_sb[:, u0:u1].rearrange("a (m l) -> a m l", l=NK),
            )

        with nc.Block() as block:

            @block.sync
            def _(sync):
                for c in range(NCH):
                    sync.dma_start(
                        wt[c][:, :], wbuf[:, c * CHUNK_W : (c + 1) * CHUNK_W]
                    ).then_inc(sin[c], 16)
                    sync.dma_start(
                        wt8[c][:, :], wbuf8[:, c * CHUNK8_W : (c + 1) * CHUNK8_W]
                    ).then_inc(sin8[c], 16)
                # out DMAs: fine units early (g0 quarters), halves after
                n_act = 0
                for g, u0, u1 in UNITS:
                    pr, p = GROUPS[g]
                    n_act += 1
                    sync.wait_ge(sact, n_act)
                    sync.dma_start(
                        out[pr * 128 : (pr + 1) * 128, p * Q + u0 : p * Q + u1],
                        o_sb[g][:, u0:u1],
                    ).then_inc(sout, 16)

            @block.scalar
            def _(scalar):
                scalar.dma_start(b_sb[:, :], bias2[:, :]).then_inc(sTb, 16)
                scalar.dma_start(id_sb[:, :], ident[:, :]).then_inc(sid, 16)
                # delay the 1MB T broadcast until chunk0 lands so it does
                # not compete with the mm1-critical input stream
                scalar.wait_ge(sin8[0], 16)
                scalar.dma_start(
                    T_sb[:, :], trow[:, :].to_broadcast((128, Q))
                ).then_inc(sTa, 16)
                scalar.wait_ge(sTb, 16)
                n_tt = 0
                for g, u0, u1 in UNITS:
                    n_tt += 1
                    scalar.wait_ge(stt, n_tt)
                    scalar.activation(
                        o_sb[g][:, u0:u1], tmp[g][:, u0:u1],
                        mybir.ActivationFunctionType.Relu, bias=b_sb[:, :],
                    ).then_inc(sact, 1)

            @block.tensor
            def _(tensor):
                xo = 2 * KTC * OSH          # xh offset in bf16 chunk
                x8o = KTC * OSH             # xl9 offset in fp8 chunk
                n_main = NCH * KTC * 2
                n_corr = NCH * KTC
                i_main = i_corr = 0
                for chk in range(NCH):
                    tensor.wait_ge(sin[chk], 16)
                    tensor.wait_ge(sin8[chk], 16)
                    for kt in range(KTC):
                        xh_t = wt[chk][:, xo + kt * W260 : xo + (kt + 1) * W260]
                        for c_off in (0, KTC * OSH):
                            tensor.matmul(
                                h_ps[:, :],
                                wt[chk][
                                    :, c_off + kt * OSH : c_off + (kt + 1) * OSH
                                ],
                                xh_t,
                                start=(i_main == 0),
                                stop=(i_main == n_main - 1),
                            )
                            i_main += 1
                        mm = tensor.matmul(
                            corr_ps[:, :],
                            wt8[chk][:, kt * OSH : (kt + 1) * OSH].bitcast(F8),
                            wt8[chk][
                                :, x8o + kt * W260 : x8o + (kt + 1) * W260
                            ].bitcast(F8),
                            start=(i_corr == 0),
                            stop=(i_corr == n_corr - 1),
                        )
                        if i_corr == n_corr - 1:
                            mm.then_inc(spe, 1)
                        i_corr += 1
                # redistribute h (64, [z|pr|r]) -> h2 (z*64+o', pr*65+r) with
                # identity matmuls on the (already warm) PE: no DMA receipt.
                tensor.wait_ge(sid, 16)
                for z in range(2):
                    tensor.wait_ge(scp, z + 1)
                    tensor.matmul(
                        h2_ps[z * OSH : (z + 1) * OSH, :],
                        id_sb[:, :],
                        h_sb[:, z * 2 * R : (z + 1) * 2 * R],
                        start=True,
                        stop=True,
                    ).then_inc(sh2, 1)

            @block.vector
            def _(vector):
                vector.wait_ge(spe, 1)
                for z in range(2):
                    sl = slice(z * 2 * R, (z + 1) * 2 * R)
                    vector.tensor_copy(h_sb[:, sl], h_ps[:, sl]).then_inc(scpa, 1)
                for z in range(2):
                    sl = slice(z * 2 * R, (z + 1) * 2 * R)
                    vector.wait_ge(scpa, z + 1)
                    vector.scalar_tensor_tensor(
                        h_sb[:, sl], corr_ps[:, sl], 2.0 ** -13, h_sb[:, sl],
                        mybir.AluOpType.mult, mybir.AluOpType.add,
                    ).then_inc(scp, 1)
                vector.wait_ge(sh2, 2)
                vector.wait_ge(sTa, 16)
                for g, u0, u1 in UNITS:
                    o, i0, i1 = exp_aps_unit(g, u0, u1)
                    vector.tensor_tensor(
                        o, i0, i1, mybir.AluOpType.mult
                    ).then_inc(stt, 1)

    nc.compile()
    return nc


def _split_bf16(a):
    hi = a.astype(ml_dtypes.bfloat16)
    lo = (a - hi.astype(np.float32)).astype(ml_dtypes.bfloat16)
    return hi, lo


def _host_prepare(x, stft_w, conv_w, conv_b):
    """Build per-core input maps."""
    x = np.ascontiguousarray(x, dtype=np.float32)
    xp = np.zeros((B, C, 2 * Q + K), dtype=np.float32)  # padded to 4160
    xp[:, :, :S] = x
    sb_, sc_, ss_ = xp.strides
    win = np.lib.stride_tricks.as_strided(
        xp, shape=(B, C, Q, R), strides=(sb_, sc_, 2 * ss_, ss_)
    )
    Xf = win.reshape(B, C * Q, R)                      # (4, 4096, 65), b=2*pr+z
    # layout [p, kt, z, pr, r]: batch order (z,pr) -> b = [0, 2, 1, 3]
    X5 = np.ascontiguousarray(
        Xf[[0, 2, 1, 3]].reshape(2, 2, KT, 128, R).transpose(3, 2, 0, 1, 4)
    ).reshape(128, KT, W260)
    xh, xl = _split_bf16(X5)
    xl9 = np.ascontiguousarray(
        (xl.astype(np.float32) * XSC)
    ).astype(ml_dtypes.float8_e4m3)

    trow = np.ascontiguousarray(stft_w.T, dtype=np.float32).reshape(1, Q)

    in_maps = []
    for i in range(NCORES):
        cw_sh = conv_w[i * OSH : (i + 1) * OSH, :]     # (64, 4096)
        cwt = np.ascontiguousarray(
            cw_sh.reshape(OSH, KT, 128).transpose(2, 1, 0)  # (128, 32, 64)
        )
        ch, cl = _split_bf16(cwt)
        c8 = (ch.astype(np.float32) * CSC).astype(ml_dtypes.float8_e4m3)
        wbuf = np.empty((128, NCH, CHUNK_W), dtype=ml_dtypes.bfloat16)
        wbuf8 = np.empty((128, NCH, CHUNK8_W), dtype=ml_dtypes.float8_e4m3)
        for chk in range(NCH):
            sl = slice(chk * KTC, (chk + 1) * KTC)
            wbuf[:, chk, :] = np.concatenate(
                [p_[:, sl].reshape(128, -1) for p_ in (ch, cl, xh)], axis=1
            )
            wbuf8[:, chk, :] = np.concatenate(
                [p_[:, sl].reshape(128, -1) for p_ in (c8, xl9)], axis=1
            )
        bias2 = np.ascontiguousarray(
            np.tile(conv_b[i * OSH : (i + 1) * OSH], 2).reshape(128, 1),
            dtype=np.float32,
        )
        in_maps.append(
            {
                "wbuf": wbuf.reshape(128, NCH * CHUNK_W),
                "wbuf8": wbuf8.reshape(128, NCH * CHUNK8_W).view(np.uint8),
                "trow": trow,
                "bias2": bias2,
                "ident": np.eye(OSH, dtype=np.float32),
            }
        )
    return in_maps


def kernel(x, stft_w, conv_w, conv_b):
    global _PROGRAM, _LAST_RESULTS
    if _PROGRAM is None:
        _PROGRAM = _build_program()
    in_maps = _host_prepare(
        np.asarray(x), np.asarray(stft_w, dtype=np.float32),
        np.asarray(conv_w, dtype=np.float32), np.asarray(conv_b, dtype=np.float32),
    )
    res = run_bass_kernel_spmd(_PROGRAM, in_maps, list(range(NCORES)))
    _LAST_RESULTS = res
    # per-core out: (256, 4096); rows pr*128 + z*64 + o' -> (b=2*pr+z, o=i*64+o')
    full = np.empty((B, OUT, S), dtype=np.float32)
    for i in range(NCORES):
        full[:, i * OSH : (i + 1) * OSH, :] = res.results[i]["out"].reshape(
            B, OSH, S
        )
    return full


if __name__ == "__main__":
    rng = np.random.default_rng(0)
    out = kernel(
        rng.standard_normal((B, C, S), dtype=np.float32),
        rng.standard_normal((NK, K), dtype=np.float32),
        (rng.standard_normal((OUT, C * K * NK)) * 0.02).astype(np.float32),
        (rng.standard_normal((OUT,)) * 0.02).astype(np.float32),
    )
    print(out.shape, out.dtype, float(np.abs(out).max()))
